# revision 3
# baseline (speedup 1.0000x reference)
"""MultiHeadAttention TRN2 kernel: B=2, S=2048, E=1024, H=16, D=64.

Sharding: 8 cores = 2 batches x 4 head-groups (4 heads / 256 channels each).
Each core computes a partial output [2048, 1024] (its heads' contribution to
the final projection); the host sums the 4 partials per batch.
"""

import sys

sys.path.insert(0, "/opt/trn_rl_repo")

import numpy as np

B, S, E, H, D = 2, 2048, 1024, 16, 64
HG = 4            # head-groups (cores per batch)
HPG = H // HG     # heads per core = 4
CG = HPG * D      # channels per core = 256
P = 128
NCORES = 8

_CACHED = {}
LAST_RESULT = None


def _build_nc():
    import concourse.bass as bass  # noqa: F401
    import concourse.mybir as mybir
    import concourse.tile as tile
    from concourse import bacc
    from concourse.masks import make_identity

    f32 = mybir.dt.float32
    f32r = mybir.dt.float32r
    AF = mybir.ActivationFunctionType

    nc = bacc.Bacc("TRN2", target_bir_lowering=False, debug=False)

    xq = nc.dram_tensor("xq", [S, E], f32, kind="ExternalInput")
    xk = nc.dram_tensor("xk", [S, E], f32, kind="ExternalInput")
    xv = nc.dram_tensor("xv", [S, E], f32, kind="ExternalInput")
    wq = nc.dram_tensor("wq", [E, CG], f32, kind="ExternalInput")
    wk = nc.dram_tensor("wk", [E, CG], f32, kind="ExternalInput")
    wv = nc.dram_tensor("wv", [E, CG], f32, kind="ExternalInput")
    wo = nc.dram_tensor("wo", [CG, E], f32, kind="ExternalInput")
    bq_d = nc.dram_tensor("bq", [CG], f32, kind="ExternalInput")
    bk_d = nc.dram_tensor("bk", [CG], f32, kind="ExternalInput")
    bv_d = nc.dram_tensor("bv", [CG], f32, kind="ExternalInput")
    bo_d = nc.dram_tensor("bo", [E], f32, kind="ExternalInput")
    out_d = nc.dram_tensor("out", [S, E], f32, kind="ExternalOutput")

    EO = E // P       # 8 e-subtiles
    SC = 4            # S-chunks of 512 in phase 1
    SCW = S // SC     # 512
    S4 = SCW // P     # 4 s-subtiles per chunk
    SO = S // P       # 16 sk-chunks in phase 2
    MC = CG // P      # 2 m-chunks of head-channels

    with tile.TileContext(nc) as tc:
        with tc.tile_pool(name="persist", bufs=1) as pers:
            # ---- static tiles (whole kernel lifetime)
            wq_sb = pers.tile([P, EO, CG], f32r)
            wk_sb = pers.tile([P, EO, CG], f32r)
            wv_sb = pers.tile([P, EO, CG], f32r)
            wo_sb = pers.tile([P, MC, E], f32r)
            nc.sync.dma_start(wq_sb[:], wq.rearrange("(ko ki) c -> ki ko c", ki=P).bitcast(f32r))
            nc.sync.dma_start(wk_sb[:], wk.rearrange("(ko ki) c -> ki ko c", ki=P).bitcast(f32r))
            nc.sync.dma_start(wv_sb[:], wv.rearrange("(ko ki) c -> ki ko c", ki=P).bitcast(f32r))
            nc.sync.dma_start(wo_sb[:], wo.rearrange("(ho hi) e -> hi ho e", hi=P).bitcast(f32r))

            bq_sb = pers.tile([P, MC], f32)
            bk_sb = pers.tile([P, MC], f32)
            nc.sync.dma_start(bq_sb[:], bq_d.rearrange("(mc p) -> p mc", p=P))
            nc.sync.dma_start(bk_sb[:], bk_d.rearrange("(mc p) -> p mc", p=P))
            bq8_sb = pers.tile([P, MC], f32)
            nc.vector.tensor_scalar_mul(bq8_sb[:], bq_sb[:], 0.125)

            bv_row = pers.tile([1, CG], f32)
            bo_row = pers.tile([1, E], f32)
            nc.sync.dma_start(bv_row[:], bv_d[None, :])
            nc.sync.dma_start(bo_row[:], bo_d[None, :])

            ident32 = pers.tile([P, P], f32)
            make_identity(nc, ident32[:])
            identr = pers.tile([P, P], f32r)
            nc.vector.tensor_copy(identr[:], ident32[:])

            ones_row = pers.tile([1, P], f32)
            nc.gpsimd.memset(ones_row[:], 1.0)
            ones_col = pers.tile([P, 1], f32)
            nc.gpsimd.memset(ones_col[:], 1.0)

            # broadcast bv/bo across partitions (scoped psum pool)
            bv_bc = pers.tile([P, CG], f32)
            bo_bc = pers.tile([P, E], f32)
            with tc.tile_pool(name="psum_setup", bufs=1, space="PSUM") as psum_su:
                pb1 = psum_su.tile([P, CG], f32, tag="su")
                nc.tensor.matmul(pb1[:], ones_row[:], bv_row[:], start=True, stop=True)
                nc.vector.tensor_copy(bv_bc[:], pb1[:])
                for i in range(2):
                    pb2 = psum_su.tile([P, 512], f32, tag="su2")
                    nc.tensor.matmul(pb2[:], ones_row[:], bo_row[:, i * 512:(i + 1) * 512], start=True, stop=True)
                    nc.vector.tensor_copy(bo_bc[:, i * 512:(i + 1) * 512], pb2[:])

            # persistent activation tensors
            qT = [pers.tile([P, S], f32r, tag=f"qT{m}", name=f"qT{m}") for m in range(MC)]
            kT = [pers.tile([P, S], f32r, tag=f"kT{m}", name=f"kT{m}") for m in range(MC)]
            v_sb = pers.tile([P, SO, HPG, D + 1], f32r)
            outT = [pers.tile([P, S], f32r, tag=f"oT{m}", name=f"oT{m}") for m in range(MC)]

            # ones column of v_aug
            for so in range(SO):
                nc.vector.tensor_copy(
                    v_sb[:, so, :, D:D + 1],
                    ones_col[:, None, :].to_broadcast((P, HPG, 1)),
                )

            # ================= Phase 1: transpose + QKV projections =========
            with (
                tc.tile_pool(name="p1", bufs=2) as p1,
                tc.tile_pool(name="psum_t", bufs=2, space="PSUM") as psum_t,
                tc.tile_pool(name="psum_p", bufs=2, space="PSUM") as psum_p,
            ):
                for which, src in (("k", xk), ("v", xv), ("q", xq)):
                    for sc in range(SC):
                        x_sb = p1.tile([P, S4, E], f32r, tag="xin")
                        nc.sync.dma_start(
                            x_sb[:],
                            src[sc * SCW:(sc + 1) * SCW]
                            .rearrange("(s4 si) e -> si s4 e", si=P)
                            .bitcast(f32r),
                        )
                        xt = p1.tile([P, EO, SCW], f32r, tag="xt")
                        for eo in range(EO):
                            pt = psum_t.tile([P, SCW], f32, tag="pt")
                            for s4 in range(S4):
                                nc.tensor.transpose(
                                    pt.bitcast(f32r)[:, s4 * P:(s4 + 1) * P],
                                    x_sb[:, s4, eo * P:(eo + 1) * P],
                                    identr[:],
                                )
                            if eo % 2 == 0:
                                nc.vector.tensor_copy(xt[:, eo, :], pt[:])
                            else:
                                nc.scalar.activation(xt[:, eo, :], pt[:], AF.Identity, scale=1.0)

                        if which in ("q", "k"):
                            w_sb = wq_sb if which == "q" else wk_sb
                            dstT = qT if which == "q" else kT
                            bias = bq8_sb if which == "q" else bk_sb
                            scl = 0.125 if which == "q" else 1.0
                            for mc in range(MC):
                                pp = psum_p.tile([P, SCW], f32, tag="pp")
                                for eo in range(EO):
                                    nc.tensor.matmul(
                                        pp[:],
                                        w_sb[:, eo, mc * P:(mc + 1) * P],
                                        xt[:, eo, :],
                                        start=(eo == 0),
                                        stop=(eo == EO - 1),
                                    )
                                nc.scalar.activation(
                                    dstT[mc][:, sc * SCW:(sc + 1) * SCW],
                                    pp[:],
                                    AF.Identity,
                                    bias=bias[:, mc:mc + 1],
                                    scale=scl,
                                )
                        else:  # v: natural layout [s, channels]
                            for s4 in range(S4):
                                pv = psum_p.tile([P, SCW], f32, tag="pp")
                                for eo in range(EO):
                                    nc.tensor.matmul(
                                        pv[:, :CG],
                                        xt[:, eo, s4 * P:(s4 + 1) * P],
                                        wv_sb[:, eo, :],
                                        start=(eo == 0),
                                        stop=(eo == EO - 1),
                                    )
                                so = sc * S4 + s4
                                nc.vector.tensor_add(
                                    v_sb[:, so, :, 0:D],
                                    pv[:, :CG].rearrange("p (h d) -> p h d", h=HPG),
                                    bv_bc.rearrange("p (h d) -> p h d", h=HPG),
                                )

            # ================= Phase 2: attention per head ==================
            with (
                tc.tile_pool(name="p2", bufs=3) as p2,
                tc.tile_pool(name="psum_qk", bufs=2, space="PSUM") as psum_qk,
                tc.tile_pool(name="psum_pv", bufs=4, space="PSUM") as psum_pv,
            ):
                for h in range(HPG):
                    mcq = h // 2
                    off = (h % 2) * D
                    pv_ps = [psum_pv.tile([D + 1, 512], f32, tag="pv", name=f"pv{h}_{i}") for i in range(4)]
                    for so in range(SO):
                        for half in range(2):
                            qk = psum_qk.tile([P, 1024], f32, tag="qk")
                            for sq2 in range(2):
                                nc.tensor.matmul(
                                    qk[:, sq2 * 512:(sq2 + 1) * 512],
                                    kT[mcq][off:off + D, so * P:(so + 1) * P],
                                    qT[mcq][off:off + D, half * 1024 + sq2 * 512: half * 1024 + (sq2 + 1) * 512],
                                    start=True,
                                    stop=True,
                                )
                            at = p2.tile([P, 1024], f32r, tag="at")
                            nc.scalar.activation(at[:], qk[:], AF.Exp, scale=1.0)
                            for sq2 in range(2):
                                sq = half * 2 + sq2
                                nc.tensor.matmul(
                                    pv_ps[sq][:],
                                    v_sb[:, so, h, :],
                                    at[:, sq2 * 512:(sq2 + 1) * 512],
                                    start=(so == 0),
                                    stop=(so == SO - 1),
                                )
                    # finalize head: divide by softmax denominator (row D)
                    for sq in range(4):
                        rrow = p2.tile([1, 512], f32, tag="rrow")
                        nc.vector.reciprocal(rrow[:], pv_ps[sq][D:D + 1, :])
                        bc = psum_qk.tile([P, 1024], f32, tag="qk")
                        nc.tensor.matmul(bc[:D, :512], ones_row[:, :D], rrow[:], start=True, stop=True)
                        rec_sb = p2.tile([D, 512], f32, tag="rec")
                        nc.scalar.activation(rec_sb[:], bc[:D, :512], AF.Identity, scale=1.0)
                        nc.vector.tensor_mul(
                            outT[mcq][off:off + D, sq * 512:(sq + 1) * 512],
                            pv_ps[sq][0:D, :],
                            rec_sb[:],
                        )

            # ================= Phase 3: output projection ===================
            with (
                tc.tile_pool(name="p3", bufs=2) as p3,
                tc.tile_pool(name="psum_o", bufs=2, space="PSUM") as psum_o,
            ):
                out_r = out_d.rearrange("(so si) e -> so si e", si=P)
                for so in range(SO):
                    po = psum_o.tile([P, E], f32, tag="po")
                    for ec in range(2):
                        for ho in range(MC):
                            nc.tensor.matmul(
                                po[:, ec * 512:(ec + 1) * 512],
                                outT[ho][:, so * P:(so + 1) * P],
                                wo_sb[:, ho, ec * 512:(ec + 1) * 512],
                                start=(ho == 0),
                                stop=(ho == MC - 1),
                            )
                    o_sb = p3.tile([P, E], f32, tag="osb")
                    nc.vector.tensor_add(o_sb[:], po[:], bo_bc[:])
                    nc.sync.dma_start(out_r[so], o_sb[:])

    nc.compile()
    return nc


def kernel(query, key, value, Wq, bq, Wk, bk, Wv, bv, Wo, bo):
    global LAST_RESULT
    from concourse.bass_utils import run_bass_kernel_spmd

    if "nc" not in _CACHED:
        _CACHED["nc"] = _build_nc()
    nc = _CACHED["nc"]

    query = np.ascontiguousarray(np.asarray(query, dtype=np.float32))
    key = np.ascontiguousarray(np.asarray(key, dtype=np.float32))
    value = np.ascontiguousarray(np.asarray(value, dtype=np.float32))
    Wq = np.asarray(Wq, dtype=np.float32)
    Wk = np.asarray(Wk, dtype=np.float32)
    Wv = np.asarray(Wv, dtype=np.float32)
    Wo = np.asarray(Wo, dtype=np.float32)
    bq = np.asarray(bq, dtype=np.float32)
    bk = np.asarray(bk, dtype=np.float32)
    bv = np.asarray(bv, dtype=np.float32)
    bo = np.asarray(bo, dtype=np.float32)

    in_maps = []
    for c in range(NCORES):
        b = c // HG
        g = c % HG
        cs = slice(g * CG, (g + 1) * CG)
        in_maps.append({
            "xq": query[b],
            "xk": key[b],
            "xv": value[b],
            "wq": np.ascontiguousarray(Wq[:, cs]),
            "wk": np.ascontiguousarray(Wk[:, cs]),
            "wv": np.ascontiguousarray(Wv[:, cs]),
            "wo": np.ascontiguousarray(Wo[cs, :]),
            "bq": np.ascontiguousarray(bq[cs]),
            "bk": np.ascontiguousarray(bk[cs]),
            "bv": np.ascontiguousarray(bv[cs]),
            "bo": bo,
        })

    res = run_bass_kernel_spmd(nc, in_maps, list(range(NCORES)))
    LAST_RESULT = res

    out = np.empty((B, S, E), dtype=np.float32)
    for b in range(B):
        acc = np.zeros((S, E), dtype=np.float64)
        for g in range(HG):
            acc += res.results[b * HG + g]["out"].astype(np.float64)
        out[b] = acc.astype(np.float32)
    return out
